# revision 18
# baseline (speedup 1.0000x reference)
"""Trainium2 Bass kernel for a 4-layer transformer decoder.

Sharding: DP2 x TP4 with 2-chunk software pipelining.
  - Cores 0-3 process batch 0, cores 4-7 batch 1.  Within each group of
    4 cores: tensor-parallel over 4 heads/core, 1024/4096 FFN hidden per
    core, 8000/32000 vocab columns per core.
  - Each sublayer boundary AllReduce ([D, 512] bf16) is split into two
    256-token chunks so the collective of chunk 0 overlaps chunk 1's
    compute, and the next sublayer's chunk-0 compute starts as soon as
    its AllReduce lands.  24 chunk-ARs of 0.5 MB (~20us each in-kernel)
    pipelined against compute of the same order.
  - Causal self-attention: chunk 0 (tokens 0-255) only touches key
    blocks 0-1; fully-masked key blocks are skipped.

Carried over from the TP8 predecessor:
  - Embedding gather + positional add on host; activations feature-major
    ([D, T]); BN folded into weights host-side (z = (x - gamma)/sigma
    tracked affinely); boundary = one DVE scalar_tensor_tensor per tile.
  - Residual z in fp16 end-to-end (backend forbids mixing f32r with
    16-bit matmul inputs); qkv/ffn1/vocab weights fp16, rest bf16; the
    vocab matmul consumes z directly (no bf16 shadow).
  - Softmax in [k, q] layout; denominator via ones column in the AV
    stationary operand; causal mask only on diagonal 128x128 blocks.
  - Next-layer weight DMA hoisted right after the current layer's last
    use of each single-buffered pool.
"""

import sys
import numpy as np

if "/opt/trn_rl_repo" not in sys.path:
    sys.path.insert(0, "/opt/trn_rl_repo")

import ml_dtypes
import concourse.bass as bass
import concourse.mybir as mybir
import concourse.tile as tile
from concourse import bacc
from concourse import bass_utils

# model dims (hardcoded per spec)
V, D, H, L, B, S, SE = 32000, 1024, 16, 4, 2, 512, 512
DH = D // H
EPS = 1e-3
NC = 8                 # cores
GRP = 4                # tensor-parallel group size
HL = H // GRP          # 4 heads per core
EB = 2                 # head tiles per core (2 heads per 128-row tile)
EL = HL * DH           # 256 local head dims
FF = 4 * D             # 4096
FFL = FF // GRP        # 1024
VL = V // GRP          # 8000
VPAD = 8192            # padded vocab shard
TL = S                 # 512 local tokens (one batch per core)
CH = 2                 # token chunks for AR pipelining
CW = TL // CH          # 256
DT = D // 128          # 8 d-tiles
HT = FFL // 128        # 8 ffn tiles per core
KB = TL // 128         # 4 key blocks
VS = VPAD // 128       # 64 vocab slices per core
NBND = 3 * L           # 12 boundaries

F32R = mybir.dt.float32r
F32 = mybir.dt.float32
BF16 = mybir.dt.bfloat16
FP16 = mybir.dt.float16
AF = mybir.ActivationFunctionType
OP = mybir.AluOpType

# bias-tile column layout
COL_QKV = 0                      # L*6*EB cols: (l*6 + which*3 + p)*EB + eb
COL_B1 = COL_QKV + 6 * L * EB    # L*HT cols: l*HT + ht
COL_SIG = COL_B1 + HT * L        # 12*8 cols: bnd*8 + dt
COL_BOUT = COL_SIG + 8 * NBND    # VS cols
COL_EPS = COL_BOUT + VS
NBCOL = COL_EPS + 1


def _build_program():
    nc = bacc.Bacc("TRN2", target_bir_lowering=False, debug=False,
                   num_devices=NC)
    dd = lambda name, shape, dtype=F32R, kind="ExternalInput": \
        nc.dram_tensor(name, shape, dtype, kind=kind).ap()

    xt = dd("xt", [D, TL], FP16)
    enct = dd("enct", [D, TL], BF16)
    attw_s = dd("attw_s", [L, 128, 3 * EL * DT], FP16)    # col = dt*768+p*256+eb*128
    attq_c = dd("attq_c", [L, 128, EL * DT], FP16)        # col = dt*256 + eb*128
    attkv_c = dd("attkv_c", [L, 128, 2 * EL * DT], BF16)  # dt*512+pi*256+eb*128
    wo_s = dd("wo_s", [L, EB, 128, D], BF16)
    wo_c = dd("wo_c", [L, EB, 128, D], BF16)
    w1p = dd("w1p", [L, 128, FFL * DT], FP16)             # col = dt*1024 + f
    w2p = dd("w2p", [L, 128, D * HT], BF16)         # col = ht*1024 + dout
    woutp = dd("woutp", [128, VS * D], FP16)        # col = vs*1024 + dt*128 + j
    biasp = dd("biasp", [128, NBCOL], F32)
    maskd = dd("maskd", [128, 128], BF16)           # strictly-lower 0/1
    identd = dd("identd", [128, 128], BF16)
    onesd = dd("onesd", [128, 64], BF16)
    logt = dd("logt", [VPAD, TL], F32, kind="ExternalOutput")

    RG = [list(range(GRP)), list(range(GRP, 2 * GRP))]
    from contextlib import ExitStack
    with tile.TileContext(nc) as tc, ExitStack() as _es:
        P = lambda **kw: _es.enter_context(tc.tile_pool(**kw))
        cst = P(name="cst", bufs=1)
        zp = P(name="zp", bufs=1)
        encp = P(name="encp", bufs=1)
        qkvp = P(name="qkvp", bufs=2)
        ckvp = P(name="ckvp", bufs=2)
        vap = P(name="vap", bufs=2)
        esp = P(name="esp", bufs=5)
        hdp = P(name="hdp", bufs=4)
        csp = P(name="csp", bufs=2)
        hfp = P(name="hfp", bufs=1)
        arp = P(name="arp", bufs=2)
        aop = P(name="aop", bufs=2)
        wap = P(name="wap", bufs=1)
        waqc = P(name="waqc", bufs=1)
        wakv = P(name="wakv", bufs=1)
        wop = P(name="wop", bufs=1)
        w1pool = P(name="w1pool", bufs=1)
        w2pool = P(name="w2pool", bufs=1)
        wvp = P(name="wvp", bufs=4)
        osp = P(name="osp", bufs=2)
        ps = P(name="ps", bufs=6, space="PSUM")
        pst = P(name="pst", bufs=2, space="PSUM")
        dram = P(name="dram", bufs=8, space="DRAM")

        bias_sb = cst.tile([128, NBCOL], F32)
        nc.sync.dma_start(bias_sb[:], biasp[:])
        mask_sb = cst.tile([128, 128], BF16)
        nc.sync.dma_start(mask_sb[:], maskd[:])
        ident = cst.tile([128, 128], BF16)
        nc.sync.dma_start(ident[:], identd[:])
        ones_sb = cst.tile([128, 64], BF16)
        nc.sync.dma_start(ones_sb[:], onesd[:])
        zeros_sb = cst.tile([128, 128], BF16)
        nc.vector.tensor_scalar_mul(zeros_sb[:, 0:64], ones_sb[:], 0.0)
        nc.vector.tensor_copy(zeros_sb[:, 64:128], zeros_sb[:, 0:64])

        def load_aw(l):
            t = wap.tile([128, 3 * EL * DT], FP16, name="aw")
            nc.sync.dma_start(t[:], attw_s[l])
            return t

        def load_wo(l, which):
            src = wo_s if which == 0 else wo_c
            nm = "wot" if which == 0 else "woc"
            ts = []
            for hb in range(EB):
                t = wop.tile([128, D], BF16, name=f"{nm}{hb}")
                nc.sync.dma_start(t[:], src[l, hb])
                ts.append(t)
            return ts

        def load_aqkv_c(l):
            q = waqc.tile([128, EL * DT], FP16, name="aqc")
            nc.sync.dma_start(q[:], attq_c[l])
            kv = wakv.tile([128, 2 * EL * DT], BF16, name="akvc")
            nc.sync.dma_start(kv[:], attkv_c[l])
            return q, kv

        def load_ffn(l):
            w1t = w1pool.tile([128, FFL * DT], FP16, name="w1t")
            nc.sync.dma_start(w1t[:], w1p[l])
            w2t = w2pool.tile([128, D * HT], BF16, name="w2t")
            nc.sync.dma_start(w2t[:], w2p[l])
            return w1t, w2t

        # encoder activations resident in bf16 (cross-attn k/v source)
        encs = []
        for dt in range(DT):
            et = encp.tile([128, TL], BF16, name=f"enc{dt}")
            nc.sync.dma_start(et[:], enct[dt * 128:(dt + 1) * 128, :])
            encs.append(et)

        # residual stream tiles (one local batch)
        z = [None] * DT
        for dt in range(DT):
            zt = zp.tile([128, TL], FP16, name=f"z{dt}")
            nc.sync.dma_start(zt[:], xt[dt * 128:(dt + 1) * 128, :])
            z[dt] = zt

        def bcol(c):
            return bias_sb[:, c:c + 1]

        def proj_ch(srcs, wsl, bias_base, tiles, ch):
            """tiles[eb][:, chunk] = (w block).T @ src[:, chunk] (+bias)."""
            c0 = ch * CW
            for eb in range(EB):
                pp = ps.tile([128, CW], F32, name="pp", tag="mm")
                for dt in range(DT):
                    nc.tensor.matmul(pp[:], wsl(dt, eb),
                                     srcs[dt][:, c0:c0 + CW],
                                     start=(dt == 0), stop=(dt == DT - 1))
                nc.scalar.activation(tiles[eb][:, c0:c0 + CW], pp[:],
                                     AF.Identity, bias=bcol(bias_base + eb))

        def build_vaug(vt, kb):
            """va[kb] = [v_h0^T | ones | v_h1^T] for one key block."""
            va = vap.tile([128, 192], BF16, name=f"va{kb}")
            pt = pst.tile([128, 128], BF16, name="ptr")
            nc.tensor.transpose(pt[:], vt[:, kb * 128:(kb + 1) * 128],
                                ident[:])
            nc.vector.tensor_copy(va[:, 0:64], pt[:, 0:64])
            nc.vector.tensor_copy(va[:, 128:192], pt[:, 64:128])
            nc.vector.tensor_copy(va[:, 64:128], ones_sb[:])
            return va

        def attn_ch(qt, kt, vaug, causal, ch, hd):
            """hd[:, chunk] = softmax(k^T q) AV for the 2 heads in a tile."""
            c0 = ch * CW
            kbs = list(range(2 * ch + 2)) if causal else list(range(KB))
            for h in range(2):
                es = []
                for kb in kbs:
                    pp = ps.tile([128, CW], F32, name="psc", tag="mm")
                    nc.tensor.matmul(pp[:],
                                     kt[h * 64:(h + 1) * 64,
                                        kb * 128:(kb + 1) * 128],
                                     qt[h * 64:(h + 1) * 64, c0:c0 + CW],
                                     start=True, stop=True)
                    et = esp.tile([128, CW], BF16, name="es")
                    dq = kb * 128 - c0   # local col where diag block starts
                    if causal and dq >= 0:
                        nc.scalar.activation(et[:, dq:CW], pp[:, dq:CW],
                                             AF.Exp)
                        if dq > 0:
                            nc.vector.tensor_copy(et[:, 0:dq], zeros_sb[:])
                        nc.vector.tensor_tensor(et[:, dq:dq + 128],
                                                et[:, dq:dq + 128],
                                                mask_sb[:], op=OP.mult)
                    else:
                        nc.scalar.activation(et[:], pp[:], AF.Exp)
                    es.append(et)
                po = ps.tile([128, CW], F32, name="po", tag="mm")
                for i, kb in enumerate(kbs):
                    nc.tensor.matmul(po[:], vaug[kb][:, 64 * h:64 * h + 128],
                                     es[i][:],
                                     start=(i == 0), stop=(i == len(kbs) - 1))
                nrows = po[0:64, :] if h == 0 else po[64:128, :]
                crows = po[64:128, :] if h == 0 else po[0:64, :]
                cs = csp.tile([64, CW], F32, name="cs")
                nc.scalar.activation(cs[:], crows, AF.Identity,
                                     bias=bias_sb[0:64, COL_EPS:COL_EPS + 1])
                rc = csp.tile([64, CW], F32, name="rc")
                nc.vector.reciprocal_approx_fast(out=rc[:], in_=cs[:])
                nc.vector.tensor_tensor(hd[h * 64:(h + 1) * 64, c0:c0 + CW],
                                        nrows, rc[:], op=OP.mult)

        def partial_ar(srcs, wsl, ch):
            """Chunk AllReduce(sum_i wsl(i,dout).T @ srcs[i][:, chunk])."""
            c0 = ch * CW
            arin = dram.tile([D, CW], BF16, name="arin")
            arout = dram.tile([D, CW], BF16, name="arout")
            ocw = aop.tile([128, DT * CW], BF16, name="ocw")
            nsrc = len(srcs)
            for half in range(2):
                for dout in range(half * 4, half * 4 + 4):
                    pw = ps.tile([128, CW], F32, name="pw", tag="mm")
                    for i in range(nsrc):
                        nc.tensor.matmul(pw[:], wsl(i, dout),
                                         srcs[i][:, c0:c0 + CW],
                                         start=(i == 0), stop=(i == nsrc - 1))
                    osl = ocw[:, dout * CW:(dout + 1) * CW]
                    if dout % 2 == 0:
                        nc.scalar.activation(osl, pw[:], AF.Copy)
                    else:
                        nc.vector.tensor_copy(osl, pw[:])
                h0 = half * 4
                nc.sync.dma_start(
                    arin[h0 * 128:(h0 + 4) * 128, :].rearrange(
                        "(dt p) t -> p dt t", p=128),
                    ocw[:, h0 * CW:(h0 + 4) * CW].rearrange(
                        "p (dt t) -> p dt t", t=CW))
            nc.gpsimd.collective_compute("AllReduce", OP.add,
                                         replica_groups=RG,
                                         ins=[arin[:]], outs=[arout[:]])
            return arout

        def boundary(arout, bnd, ch):
            c0 = ch * CW
            art = arp.tile([128, DT * CW], BF16, name="art")
            for half in range(2):
                h0 = half * 4
                nc.sync.dma_start(
                    art[:, h0 * CW:(h0 + 4) * CW].rearrange(
                        "p (dt t) -> p dt t", t=CW),
                    arout[h0 * 128:(h0 + 4) * 128, :].rearrange(
                        "(dt p) t -> p dt t", p=128))
            for dt in range(DT):
                nc.vector.scalar_tensor_tensor(
                    z[dt][:, c0:c0 + CW], z[dt][:, c0:c0 + CW],
                    bcol(COL_SIG + bnd * 8 + dt),
                    art[:, dt * CW:(dt + 1) * CW], OP.mult, OP.add)

        # prologue loads for layer 0
        aw = load_aw(0)
        wot = load_wo(0, 0)
        aqc, akvc = load_aqkv_c(0)
        woc = load_wo(0, 1)
        w1t, w2t = load_ffn(0)

        zsrc = z
        for l in range(L):
            # ---- self attention, chunk-pipelined ----
            wsl_q = lambda dt, eb: aw[:, dt * 768 + eb * 128:
                                      dt * 768 + eb * 128 + 128]
            wsl_k = lambda dt, eb: aw[:, dt * 768 + 256 + eb * 128:
                                      dt * 768 + 256 + eb * 128 + 128]
            wsl_v = lambda dt, eb: aw[:, dt * 768 + 512 + eb * 128:
                                      dt * 768 + 512 + eb * 128 + 128]
            qt = [qkvp.tile([128, TL], BF16, name=f"qt{eb}")
                  for eb in range(EB)]
            kt = [qkvp.tile([128, TL], BF16, name=f"kt{eb}")
                  for eb in range(EB)]
            vt = [qkvp.tile([128, TL], BF16, name=f"vt{eb}")
                  for eb in range(EB)]
            hds = [hdp.tile([128, TL], BF16, name="hd") for _ in range(EB)]
            vaug = [[None] * KB for _ in range(EB)]
            ar_s = [None] * CH
            for ch in range(CH):
                proj_ch(zsrc, wsl_k, COL_QKV + (l * 6 + 1) * EB, kt, ch)
                proj_ch(zsrc, wsl_v, COL_QKV + (l * 6 + 2) * EB, vt, ch)
                proj_ch(zsrc, wsl_q, COL_QKV + (l * 6 + 0) * EB, qt, ch)
                for hb in range(EB):
                    for kb in (2 * ch, 2 * ch + 1):
                        vaug[hb][kb] = build_vaug(vt[hb], kb)
                    attn_ch(qt[hb], kt[hb], vaug[hb], True, ch, hds[hb])
                ar_s[ch] = partial_ar(
                    hds, lambda hb, dout: wot[hb][:, dout * 128:
                                                  (dout + 1) * 128], ch)
            if l + 1 < L:
                aw_n = load_aw(l + 1)
                wot_n = load_wo(l + 1, 0)

            # cross k/v from encoder — independent of the self AllReduce;
            # fills the AR(self, c1) window on the PE.
            wsl_kc = lambda dt, eb: akvc[:, dt * 512 + eb * 128:
                                         dt * 512 + eb * 128 + 128]
            wsl_vc = lambda dt, eb: akvc[:, dt * 512 + 256 + eb * 128:
                                         dt * 512 + 256 + eb * 128 + 128]
            ktc = [ckvp.tile([128, TL], BF16, name=f"ktc{eb}")
                   for eb in range(EB)]
            vtc = [ckvp.tile([128, TL], BF16, name=f"vtc{eb}")
                   for eb in range(EB)]
            for ch in range(CH):
                proj_ch(encs, wsl_kc, COL_QKV + (l * 6 + 4) * EB, ktc, ch)
                proj_ch(encs, wsl_vc, COL_QKV + (l * 6 + 5) * EB, vtc, ch)
            vaugc = [[build_vaug(vtc[hb], kb) for kb in range(KB)]
                     for hb in range(EB)]

            # ---- cross attention, chunk-pipelined ----
            wsl_qc = lambda dt, eb: aqc[:, dt * 256 + eb * 128:
                                        dt * 256 + eb * 128 + 128]
            qtc = [qkvp.tile([128, TL], BF16, name=f"qt{eb}")
                   for eb in range(EB)]
            hdc = [hdp.tile([128, TL], BF16, name="hd") for _ in range(EB)]
            ar_c = [None] * CH
            for ch in range(CH):
                boundary(ar_s[ch], 3 * l, ch)
                proj_ch(zsrc, wsl_qc, COL_QKV + (l * 6 + 3) * EB, qtc, ch)
                for hb in range(EB):
                    attn_ch(qtc[hb], ktc[hb], vaugc[hb], False, ch, hdc[hb])
                ar_c[ch] = partial_ar(
                    hdc, lambda hb, dout: woc[hb][:, dout * 128:
                                                  (dout + 1) * 128], ch)
            if l + 1 < L:
                aqc_n, akvc_n = load_aqkv_c(l + 1)
                woc_n = load_wo(l + 1, 1)

            # ---- FFN, chunk-pipelined ----
            hts = [hfp.tile([128, TL], BF16, name=f"hf{ht}")
                   for ht in range(HT)]
            ar_f = [None] * CH
            for ch in range(CH):
                boundary(ar_c[ch], 3 * l + 1, ch)
                c0 = ch * CW
                for ht in range(HT):
                    pp = ps.tile([128, CW], F32, name="pf", tag="mm")
                    for dt in range(DT):
                        cc0 = dt * FFL + ht * 128
                        nc.tensor.matmul(pp[:], w1t[:, cc0:cc0 + 128],
                                         z[dt][:, c0:c0 + CW],
                                         start=(dt == 0), stop=(dt == DT - 1))
                    nc.scalar.activation(hts[ht][:, c0:c0 + CW], pp[:],
                                         AF.Relu,
                                         bias=bcol(COL_B1 + l * HT + ht))
                ar_f[ch] = partial_ar(
                    hts, lambda ht, dout: w2t[:, ht * D + dout * 128:
                                              ht * D + dout * 128 + 128], ch)
            if l + 1 < L:
                w1t_n, w2t_n = load_ffn(l + 1)
            if l + 1 == L:
                wpre = []
                for vs2 in (0, 2):
                    wt = wvp.tile([128, 2 * D], FP16, name="wv")
                    nc.sync.dma_start(wt[:], woutp[:, vs2 * D:(vs2 + 2) * D])
                    wpre.append(wt)
            for ch in range(CH):
                boundary(ar_f[ch], 3 * l + 2, ch)
            if l + 1 < L:
                aw, wot, aqc, akvc, woc = aw_n, wot_n, aqc_n, akvc_n, woc_n
                w1t, w2t = w1t_n, w2t_n

        zb = z  # vocab matmul reads the fp16 residual directly

        # vocab projection (2 slices per weight DMA)
        for vs2 in range(0, VS, 2):
            if vs2 < 4:
                wt = wpre[vs2 // 2]
            else:
                wt = wvp.tile([128, 2 * D], FP16, name="wv")
                nc.sync.dma_start(wt[:], woutp[:, vs2 * D:(vs2 + 2) * D])
            for sub in range(2):
                vs = vs2 + sub
                pp = ps.tile([128, TL], F32, name="pv", tag="mm")
                for dt in range(DT):
                    nc.tensor.matmul(
                        pp[:], wt[:, sub * D + dt * 128:
                                  sub * D + (dt + 1) * 128],
                        zb[dt][:], start=(dt == 0), stop=(dt == DT - 1))
                osb = osp.tile([128, TL], F32, name="osb")
                if vs % 2 == 0:
                    nc.scalar.activation(osb[:], pp[:], AF.Identity,
                                         bias=bcol(COL_BOUT + vs))
                else:
                    nc.vector.tensor_scalar_add(osb[:], pp[:],
                                                bcol(COL_BOUT + vs))
                nc.sync.dma_start(logt[vs * 128:(vs + 1) * 128, :], osb[:])
    nc.compile()
    return nc


def _host_prepare(inputs):
    """Fold BN/biases into weights, shard per core; returns per-core in_maps."""
    f = lambda a: np.asarray(a, dtype=np.float64)
    tobf = lambda a: a.astype(ml_dtypes.bfloat16)
    seq = np.asarray(inputs["sequence"])
    emb = np.asarray(inputs["emb"], dtype=np.float32)
    pes = np.asarray(inputs["pes"], dtype=np.float32)
    enc = np.asarray(inputs["encoder_out"], dtype=np.float32)

    x0 = emb[seq] + pes[None, :, :]                   # [B, S, D] fp32
    xts = [np.ascontiguousarray(x0[b].T.astype(np.float16))
           for b in range(B)]                         # [D, S] per batch
    encts = [np.ascontiguousarray(tobf(enc[b].T)) for b in range(B)]

    mask = (np.arange(128)[:, None] < np.arange(128)[None, :])
    maskd = np.ascontiguousarray(tobf(mask.astype(np.float32)))

    attw_s = np.zeros((GRP, L, 128, 3 * EL * DT), np.float16)
    attq_c = np.zeros((GRP, L, 128, EL * DT), np.float16)
    attkv_c = np.zeros((GRP, L, 128, 2 * EL * DT), ml_dtypes.bfloat16)
    wo_s_p = np.zeros((GRP, L, EB, 128, D), ml_dtypes.bfloat16)
    wo_c_p = np.zeros((GRP, L, EB, 128, D), ml_dtypes.bfloat16)
    w1pp = np.zeros((GRP, L, 128, FFL * DT), np.float16)
    w2pp = np.zeros((GRP, L, 128, D * HT), ml_dtypes.bfloat16)
    woutpp = np.zeros((GRP, 128, VS * D), np.float16)
    biaspp = np.zeros((GRP, 128, NBCOL), np.float32)

    def pack_kxm(w, ncols):
        kt = w.shape[0] // 128
        return w.reshape(kt, 128, ncols).transpose(1, 0, 2).reshape(
            128, kt * ncols)

    sig = np.ones(D)
    gam = np.zeros(D)
    for l in range(L):
        for which, (wq, bq, wk, bk, wv, bv, wo, bo, g, be, m, v) in enumerate([
            (inputs["wq_s"][l], inputs["bq_s"][l], inputs["wk_s"][l],
             inputs["bk_s"][l], inputs["wv_s"][l], inputs["bv_s"][l],
             inputs["wo_s"][l], inputs["bo_s"][l], inputs["g1"][l],
             inputs["be1"][l], inputs["m1"][l], inputs["v1"][l]),
            (inputs["wq_c"][l], inputs["bq_c"][l], inputs["wk_c"][l],
             inputs["bk_c"][l], inputs["wv_c"][l], inputs["bv_c"][l],
             inputs["wo_c"][l], inputs["bo_c"][l], inputs["g2"][l],
             inputs["be2"][l], inputs["m2"][l], inputs["v2"][l]),
        ]):
            wq, wk, wv = f(wq), f(wk), f(wv)          # [H, D, DH]
            bq, bk, bv = f(bq), f(bk), f(bv)          # [H, DH]
            wo, bo = f(wo), f(bo)
            for c in range(GRP):
                h0 = c * HL
                wql = wq[h0:h0 + HL].transpose(1, 0, 2).reshape(D, EL)
                wkl = wk[h0:h0 + HL].transpose(1, 0, 2).reshape(D, EL)
                wvl = wv[h0:h0 + HL].transpose(1, 0, 2).reshape(D, EL)
                bql = bq[h0:h0 + HL].reshape(EL)
                bkl = bk[h0:h0 + HL].reshape(EL)
                bvl = bv[h0:h0 + HL].reshape(EL)
                wq_eff = (sig[:, None] * wql) / 8.0
                bq_eff = (gam @ wql + bql) / 8.0
                if which == 0:
                    wk_eff = sig[:, None] * wkl
                    bk_eff = gam @ wkl + bkl
                    wv_eff = sig[:, None] * wvl
                    bv_eff = gam @ wvl + bvl
                    wcat = np.concatenate([wq_eff, wk_eff, wv_eff], axis=1)
                    attw_s[c, l] = pack_kxm(wcat, 3 * EL).astype(np.float16)
                else:
                    attq_c[c, l] = pack_kxm(wq_eff, EL).astype(np.float16)
                    kvcat = np.concatenate([wkl, wvl], axis=1)
                    attkv_c[c, l] = tobf(
                        pack_kxm(kvcat, 2 * EL).astype(np.float32))
                    bk_eff, bv_eff = bkl, bvl
                wo_loc = wo[c * EL:(c + 1) * EL, :]
                dst = (wo_s_p if which == 0 else wo_c_p)
                for hb in range(EB):
                    dst[c, l, hb] = tobf(
                        wo_loc[hb * 128:(hb + 1) * 128, :].astype(np.float32))
                for p, b_eff in enumerate((bq_eff, bk_eff, bv_eff)):
                    cb = COL_QKV + (l * 6 + which * 3 + p) * EB
                    for eb in range(EB):
                        biaspp[c, :, cb + eb] = \
                            b_eff[eb * 128:(eb + 1) * 128].astype(np.float32)
            bnd = 3 * l + which
            for c in range(GRP):
                for dt in range(DT):
                    biaspp[c, :, COL_SIG + bnd * 8 + dt] = \
                        sig[dt * 128:(dt + 1) * 128].astype(np.float32)
            beta = gam + bo
            s = f(g) / np.sqrt(f(v) + EPS)
            cshift = f(be) - f(m) * s
            sig = s
            gam = s * beta + cshift

        # FFN
        w1, b1 = f(inputs["w1"][l]), f(inputs["b1"][l])
        w2, b2 = f(inputs["w2"][l]), f(inputs["b2"][l])
        for c in range(GRP):
            cols = slice(c * FFL, (c + 1) * FFL)
            w1_eff = sig[:, None] * w1[:, cols]
            b1_eff = gam @ w1[:, cols] + b1[cols]
            w1pp[c, l] = pack_kxm(w1_eff, FFL).astype(np.float16)
            w2pp[c, l] = tobf(pack_kxm(w2[cols, :], D).astype(np.float32))
            for ht in range(HT):
                biaspp[c, :, COL_B1 + l * HT + ht] = \
                    b1_eff[ht * 128:(ht + 1) * 128].astype(np.float32)
        bnd = 3 * l + 2
        for c in range(GRP):
            for dt in range(DT):
                biaspp[c, :, COL_SIG + bnd * 8 + dt] = \
                    sig[dt * 128:(dt + 1) * 128].astype(np.float32)
        beta = gam + b2
        s = f(inputs["g3"][l]) / np.sqrt(f(inputs["v3"][l]) + EPS)
        cshift = f(inputs["be3"][l]) - f(inputs["m3"][l]) * s
        sig = s
        gam = s * beta + cshift

    wout, bout = f(inputs["w_out"]), f(inputs["b_out"])
    for c in range(GRP):
        wsl = np.zeros((D, VPAD))
        bsl = np.zeros(VPAD)
        cols = slice(c * VL, (c + 1) * VL)
        wsl[:, :VL] = wout[:, cols]
        bsl[:VL] = bout[cols]
        wout_eff = sig[:, None] * wsl
        bout_eff = gam @ wsl + bsl
        woutpp[c] = wout_eff.reshape(DT, 128, VS, 128).transpose(
            1, 2, 0, 3).reshape(128, VS * D).astype(np.float16)
        for vs in range(VS):
            biaspp[c, :, COL_BOUT + vs] = \
                bout_eff[vs * 128:(vs + 1) * 128].astype(np.float32)

    biaspp[:, :, COL_EPS] = 1e-30
    in_maps = []
    for c in range(NC):
        g, s = c // GRP, c % GRP
        in_maps.append({
            "xt": xts[g], "enct": encts[g],
            "attw_s": attw_s[s], "attq_c": attq_c[s], "attkv_c": attkv_c[s],
            "wo_s": wo_s_p[s], "wo_c": wo_c_p[s],
            "w1p": w1pp[s], "w2p": w2pp[s], "woutp": woutpp[s],
            "biasp": biaspp[s], "maskd": maskd,
            "identd": tobf(np.eye(128, dtype=np.float32)),
            "onesd": np.ones((128, 64), dtype=ml_dtypes.bfloat16),
        })
    return in_maps


_NC_CACHE = {}


def _get_program():
    if "nc" not in _NC_CACHE:
        _NC_CACHE["nc"] = _build_program()
    return _NC_CACHE["nc"]


def run(inputs, trace=False):
    nc = _get_program()
    in_maps = _host_prepare(inputs)
    res = bass_utils.run_bass_kernel_spmd(nc, in_maps, list(range(NC)),
                                          trace=trace)
    out = np.empty((B, S, V), dtype=np.float32)
    for g in range(B):
        full = np.concatenate(
            [res.results[g * GRP + s]["logt"][:VL, :] for s in range(GRP)],
            axis=0)                                   # [V, S]
        out[g] = full.T
    return out, res


def kernel(**inputs):
    out, _ = run(inputs)
    return out
